# revision 37
# baseline (speedup 1.0000x reference)
import os
import sys

sys.path.insert(0, "/opt/trn_rl_repo")

import numpy as np
import ml_dtypes

# ---------------------------------------------------------------- problem dims
NCORES = 8
N = 50000
E = 800000
IN_F, HID_F, OUT_F = 256, 128, 64
NEG = 0.2
EPS = 1e-16

NPC = N // NCORES            # 6250 nodes (= targets) per core
BPB = 128                    # targets per block
NB = (NPC + BPB - 1) // BPB  # 49 blocks per core
ROWS = NB * BPB              # 6272 padded rows per core slice
TBL = NCORES * ROWS          # 50176 rows in the all-gathered table
TH = 32768                   # int16 gather index threshold
WSHIFT = 8.0                 # global exp shift (cancels in normalization)
ZCLAMP = 12.0                # safety clamp on zl (real zl stays < 5.2)
PW = 512                     # phase-1 pass width (node columns)


def _householder(a):
    """Symmetric orthogonal H with (H h)[0] == (a/||a||) . h ; returns H, ||a||."""
    a = np.asarray(a, dtype=np.float64)
    d = a.shape[0]
    alpha = np.linalg.norm(a)
    u = a.copy()
    sgn = 1.0 if a[0] >= 0 else -1.0
    u[0] += sgn * alpha
    nu = np.linalg.norm(u)
    Hm = np.eye(d) - 2.0 * np.outer(u, u) / (nu * nu)
    # H @ a = -sgn*alpha*e0  =>  (H h)[0] = -sgn * (a.h)/alpha; fold sign into c
    return Hm.astype(np.float32), np.float32(-sgn * alpha)


def prep_structures(edge_index, group=2):
    """Host-side layout. Blocks are grouped; per group one lo + one hi gather
    call covers all blocks in the group (slot order: los of each block, then
    his). Static fp8 one-hot (and transposed) matrices select target lanes."""
    src = edge_index[0].astype(np.int64)
    tgt = edge_index[1].astype(np.int64)
    adj = (src // NPC) * ROWS + (src % NPC)  # row in all-gathered table

    order = np.argsort(tgt, kind="stable")
    src_a = adj[order]
    tgt_s = tgt[order]

    core_of = tgt_s // NPC
    blk_of = (tgt_s % NPC) // BPB
    rel_of = (tgt_s % NPC) % BPB
    gb = core_of * NB + blk_of
    bounds = np.searchsorted(gb, np.arange(NCORES * NB + 1))

    lo_cnt = np.zeros((NCORES, NB), dtype=np.int64)
    hi_cnt = np.zeros((NCORES, NB), dtype=np.int64)
    per_kb = {}
    for k in range(NCORES):
        for b in range(NB):
            g = k * NB + b
            s, e = bounds[g], bounds[g + 1]
            sa = src_a[s:e]
            rl = rel_of[s:e]
            m = sa < TH
            lo_o = np.argsort(sa[m], kind="stable")
            hi_o = np.argsort(sa[~m], kind="stable")
            per_kb[(k, b)] = (sa[m][lo_o], rl[m][lo_o], sa[~m][hi_o], rl[~m][hi_o])
            lo_cnt[k, b] = int(m.sum())
            hi_cnt[k, b] = int((~m).sum())

    nlo = np.maximum(1, np.ceil(lo_cnt.max(axis=0) / 128.0)).astype(np.int64)
    nhi = np.ceil(hi_cnt.max(axis=0) / 128.0).astype(np.int64)  # may be 0

    NG = (NB + group - 1) // group
    groups = []      # per group: dict(blocks, s0, nsg, lo_rng{b}, hi_rng{b})
    s_off = 0
    SRC_cols = []    # per-core built later
    for gi in range(NG):
        blks = list(range(gi * group, min((gi + 1) * group, NB)))
        lo_rng = {}
        hi_rng = {}
        off = 0
        for b in blks:
            lo_rng[b] = (off, int(nlo[b]))
            off += int(nlo[b])
        lo_tot = off
        for b in blks:
            hi_rng[b] = (off, int(nhi[b]))
            off += int(nhi[b])
        groups.append(dict(blocks=blks, s0=s_off, nsg=off, lo_tot=lo_tot,
                           lo_rng=lo_rng, hi_rng=hi_rng))
        s_off += off
    S_TOT = s_off
    NSGMAX = max(g["nsg"] for g in groups)

    # per-core slot grids in the grouped order
    SRC = np.zeros((NCORES, 128, S_TOT), dtype=np.int64)
    REL = np.full((NCORES, 128, S_TOT), -1, dtype=np.int64)
    for k in range(NCORES):
        for g in groups:
            for b in g["blocks"]:
                la, lr, ha, hr = per_kb[(k, b)]
                for (arr, rel, rng) in ((la, lr, g["lo_rng"][b]),
                                        (ha, hr, g["hi_rng"][b])):
                    off, cnt = rng
                    if cnt == 0:
                        continue
                    base = g["s0"] + off
                    full = np.zeros(cnt * 128, dtype=np.int64)
                    full[:len(arr)] = arr
                    fr = np.full(cnt * 128, -1, dtype=np.int64)
                    fr[:len(arr)] = rel
                    SRC[k, :, base:base + cnt] = full.reshape(cnt, 128).T
                    REL[k, :, base:base + cnt] = fr.reshape(cnt, 128).T

    # per-call wrapped int16 index arrays (0-pads; counts uniform across cores)
    idx_parts = [[] for _ in range(NCORES)]
    col_off = 0
    for g in groups:
        bcalls = []
        specs = []
        for b in g["blocks"]:
            off, cnt = g["lo_rng"][b]
            if cnt:
                specs.append((False, off, cnt))
        hi_tot = g["nsg"] - g["lo_tot"]
        if hi_tot:
            specs.append((True, g["lo_tot"], hi_tot))
        for (is_hi, slot0, cnt) in specs:
            n_idx = cnt * 128
            cols = n_idx // 16
            bcalls.append((col_off, cols, n_idx, is_hi, slot0, cnt))
            col_off += cols
            for k in range(NCORES):
                s0 = g["s0"] + slot0
                vals = SRC[k][:, s0:s0 + cnt].flatten(order="F")
                if is_hi:
                    vals = np.maximum(vals - TH, 0)
                w16 = vals.reshape(-1, 16).T  # [16, cols]
                idx_parts[k].append(np.tile(w16, (8, 1)).astype(np.int16))
        g["calls"] = bcalls
    eidx = [np.concatenate(idx_parts[k], axis=1) for k in range(NCORES)]

    # fp8 one-hot grids (flattened [128, S_TOT*128] uint8; 1.0 == 0x38)
    ONE = np.uint8(0x38)
    tvec = np.arange(128, dtype=np.int64)
    OH8 = []
    OHT8 = []
    for k in range(NCORES):
        rel = REL[k]                      # [128 lanes, S_TOT]
        oh = (rel[:, :, None] == tvec[None, None, :])        # [lane, s, t]
        OH8.append(np.where(oh, ONE, np.uint8(0)).reshape(128, S_TOT * 128))
        oht = np.transpose(oh, (2, 1, 0))                     # [t, s, lane]
        OHT8.append(np.ascontiguousarray(
            np.where(oht, ONE, np.uint8(0)).reshape(128, S_TOT * 128)))

    meta = dict(
        groups=groups, S_TOT=S_TOT, NSGMAX=NSGMAX, TOT_COLS=col_off,
    )
    return meta, eidx, OH8, OHT8


# ------------------------------------------------------------------ bass build
def _patch_tile_drain():
    """This walrus build supports only one sync-wait per SP TPB_CTRL
    instruction; TileContext's exit drain aggregates the whole global clock
    onto one drain. Split each wait onto its own single-wait NOP first."""
    import concourse.mybir as mybir
    import concourse.tile as tile
    from concourse.tile import ScopedClock

    if getattr(tile.TileContext, "_drain_split_patched", False):
        return

    def _split(self, tick_clock, wait_clock):
        nop0 = self.nc.sync.nop()
        wait_clock.add_sem_waits(
            nop0.ins, ScopedClock({None: tick_clock.global_clock})
        )
        si = nop0.ins.sync_info
        if si is not None and si.on_wait and len(si.on_wait) > 1:
            waits = list(si.on_wait)
            nop0.ins.sync_info = mybir.SyncInfo(
                on_wait=[waits[0]], on_update=list(si.on_update)
            )
            for w in waits[1:]:
                n = self.nc.sync.nop()
                n.ins.sync_info = mybir.SyncInfo(on_wait=[w], on_update=[])
        self.nc.sync.drain()
        self.nc.all_engine_barrier()
        popped = self.nc._tile_sem_poison_stack.pop()
        assert popped is self._sem_poison
        self.nc.clear_and_free_semaphores(list(self.sems.allocated().values()))
        self.nc.all_engine_barrier()

    tile.TileContext._drain_and_barrier = _split
    tile.TileContext._drain_split_patched = True


def _split_multi_waits(nc):
    """This walrus build rejects instructions carrying more than one sync
    wait. Move extra waits onto single-wait NOPs inserted just before the
    instruction (same engine, same per-engine position)."""
    import concourse.mybir as mybir

    ctr = [0]
    for f in nc.m.functions:
        for bb in f.blocks:
            insts = list(bb.instructions)
            out = []
            changed = False
            for ins in insts:
                si = getattr(ins, "sync_info", None)
                if si is not None and si.on_wait and len(si.on_wait) > 1:
                    waits = list(si.on_wait)
                    for w in waits[:-1]:
                        n = mybir.InstNoOp(
                            name=f"splitw-{ctr[0]}", ins=[], outs=[]
                        )
                        ctr[0] += 1
                        n.engine = ins.engine
                        n.sync_info = mybir.SyncInfo(on_wait=[w], on_update=[])
                        nc.register_instruction(n)
                        out.append(n)
                    ins.sync_info = mybir.SyncInfo(
                        on_wait=[waits[-1]], on_update=list(si.on_update)
                    )
                    changed = True
                out.append(ins)
            if changed:
                bb.instructions = out


def build_bass(meta, consts):
    import concourse.bass as bass
    import concourse.mybir as mybir
    import concourse.tile as tile
    from concourse.library_config import mlp as mlp_lib
    from concourse.tile_rust import add_dep_helper

    _patch_tile_drain()

    F32 = mybir.dt.float32
    F16 = mybir.dt.float16
    F8 = mybir.dt.float8e4
    I16 = mybir.dt.int16
    AL = mybir.AluOpType
    AF = mybir.ActivationFunctionType
    AX = mybir.AxisListType
    S_TOT = meta["S_TOT"]
    NSGMAX = meta["NSGMAX"]
    groups = meta["groups"]

    nc = bass.Bass(num_devices=NCORES, num_swdge_queues=4)

    # per-core external inputs
    xT_sl = nc.dram_tensor("xT_sl", [IN_F, ROWS], F16, kind="ExternalInput")
    eidx = nc.dram_tensor(
        "eidx", [128, meta["TOT_COLS"]], I16, kind="ExternalInput"
    )
    eoh = nc.dram_tensor("eoh", [128, S_TOT * 128], F8, kind="ExternalInput")
    eoht = nc.dram_tensor("eoht", [128, S_TOT * 128], F8, kind="ExternalInput")
    out_fin = nc.dram_tensor("out_fin", [ROWS, OUT_F], F32, kind="ExternalOutput")

    def inl(name, arr):
        return nc.inline_tensor(np.ascontiguousarray(arr), name=name)

    f16 = np.float16
    c_W1a = inl("W1a", consts["W1"][:128].astype(f16))
    c_W1b = inl("W1b", consts["W1"][128:].astype(f16))
    c_R1 = inl("R1", consts["R1"].astype(f16))
    c_W2 = inl("W2", consts["W2"].astype(f16))
    c_R2 = inl("R2", consts["R2"].astype(f16))
    c_aw1t = inl("aw1t", consts["a1_w"][HID_F:].astype(f16).reshape(HID_F, 1))
    c_aw2t = inl("aw2t", consts["a2_w"][OUT_F:].astype(f16).reshape(OUT_F, 1))
    c_b1 = inl("b1c", consts["b1"].astype(np.float32).reshape(HID_F, 1))
    c_b1n = inl("b1n", (-consts["b1"]).astype(np.float32).reshape(HID_F, 1))
    c_b2 = inl("b2c", consts["b2"].astype(np.float32).reshape(OUT_F, 1))
    c_ones1 = inl("ones1", np.ones((1, 1), f16))
    c_onesr = inl("onesr", np.ones((1, 128), f16))
    c_I128 = inl("I128", np.eye(128, dtype=f16))
    c_b1a = inl("b1a", np.full((1, 1), consts["a1_b"][0], np.float32))
    c_b2a = inl("b2a", np.full((1, 1), consts["a2_b"][0], np.float32))
    c_nsh = inl("nsh", np.full((128, 1), -WSHIFT, np.float32))
    c_clamp = inl("clamp", np.full((128, 1), ZCLAMP, f16))
    c1 = float(consts["c1"])
    c2 = float(consts["c2"])

    # internal DRAM
    h1p_sl = nc.dram_tensor("h1p_sl", [ROWS, 128], F16)
    h1p_all = nc.dram_tensor("h1p_all", [TBL, 128], F16, addr_space="Shared")
    h2p_sl = nc.dram_tensor("h2p_sl", [ROWS, 128], F16)
    h2p_all = nc.dram_tensor("h2p_all", [TBL, 128], F16, addr_space="Shared")

    def mid_bcast(ap, cs):  # [128, X] -> [128, cs(bcast), X]
        return bass.AP(
            ap.tensor, ap.offset, [list(ap.ap[0]), [0, cs], list(ap.ap[1])]
        )

    with tile.TileContext(nc) as tc:
        import contextlib

        with contextlib.ExitStack() as ctx:
            cpool = ctx.enter_context(tc.tile_pool(name="consts", bufs=1))
            persist = ctx.enter_context(tc.tile_pool(name="persist", bufs=1))
            sbx = ctx.enter_context(tc.tile_pool(name="sbx", bufs=3))
            sb = ctx.enter_context(tc.tile_pool(name="sb", bufs=2))
            sb3 = ctx.enter_context(tc.tile_pool(name="sb3", bufs=3))
            ps = ctx.enter_context(tc.tile_pool(name="ps", bufs=4, space="PSUM"))
            psa = ctx.enter_context(tc.tile_pool(name="psa", bufs=2, space="PSUM"))
            pst = ctx.enter_context(tc.tile_pool(name="pst", bufs=2, space="PSUM"))

            def cload(handle, shape, dtype):
                t = cpool.tile(shape, dtype, tag=handle.name)
                nc.sync.dma_start(out=t[:], in_=handle[:, :])
                return t

            W1a = cload(c_W1a, [128, 128], F16)
            W1b = cload(c_W1b, [128, 128], F16)
            R1 = cload(c_R1, [128, 128], F16)
            W2 = cload(c_W2, [128, 64], F16)
            R2 = cload(c_R2, [64, 64], F16)
            aw1t = cload(c_aw1t, [128, 1], F16)
            aw2t = cload(c_aw2t, [64, 1], F16)
            b1c = cload(c_b1, [128, 1], F32)
            b1n = cload(c_b1n, [128, 1], F32)
            b2c = cload(c_b2, [64, 1], F32)
            ones1 = cload(c_ones1, [1, 1], F16)
            onesr = cload(c_onesr, [1, 128], F16)
            I128 = cload(c_I128, [128, 128], F16)
            b1a = cload(c_b1a, [1, 1], F32)
            b2a = cload(c_b2a, [1, 1], F32)
            nsh = cload(c_nsh, [128, 1], F32)
            clampc = cload(c_clamp, [128, 1], F16)

            t1T = persist.tile([128, NB], F16, tag="t1T")
            t2T = persist.tile([128, NB], F16, tag="t2T")
            ted1 = persist.tile([128, S_TOT], F16, tag="ted1")
            ted2 = persist.tile([128, S_TOT], F16, tag="ted2")

            ll = nc.gpsimd.load_library(mlp_lib)
            nidx_regs = {}
            for grp in groups:
                for (_o, _c, _n, _h, _s0, _cnt) in grp["calls"]:
                    if _n not in nidx_regs:
                        _r = nc.gpsimd.alloc_register(f"nidx_{_n}")
                        nc.gpsimd.reg_mov(_r, _n)
                        nidx_regs[_n] = _r

            # zero the gather-destination pool once (stale-lane safety)
            gz = []
            for i in range(4):
                t = sb3.tile([128, NSGMAX, 128], F16, tag="g", bufs=4)
                nc.vector.memset(t[:, :, :], 0.0)
                gz.append(t)

            # ---------------- t_ed precompute (per layer) ----------------
            ted_cursor = {1: 0, 2: 0}

            def ted_emit(which, tT, ted, max_block_excl):
                cur = ted_cursor[which]
                while (cur < len(groups)
                       and max(groups[cur]["blocks"]) < max_block_excl):
                    grp = groups[cur]
                    nsg = grp["nsg"]
                    sg0 = grp["s0"]
                    oht = sb.tile([128, NSGMAX, 128], F8, tag="oht")
                    nc.sync.dma_start(
                        out=oht[:, 0:nsg, :],
                        in_=eoht[:, sg0 * 128:(sg0 + nsg) * 128],
                    )
                    tp = ps.tile([128, NSGMAX], F32, tag="pp")
                    for b in grp["blocks"]:
                        for (off, cnt) in (grp["lo_rng"][b], grp["hi_rng"][b]):
                            for j in range(off, off + cnt):
                                nc.tensor.matmul(
                                    tp[:, j:j + 1], lhsT=oht[:, j, :],
                                    rhs=tT[:, b:b + 1], start=True, stop=True,
                                    skip_group_check=True,
                                )
                    nc.scalar.copy(ted[:, sg0:sg0 + nsg], tp[:, 0:nsg])
                    cur += 1
                ted_cursor[which] = cur


            # ---------------- phase 1: h1' table + t1 ----------------
            for p in range((ROWS + PW - 1) // PW):
                c0 = p * PW
                W = min(PW, ROWS - c0)
                nbk = W // 128
                xa = sbx.tile([128, PW], F16, tag="xa")
                xb = sbx.tile([128, PW], F16, tag="xb")
                nc.sync.dma_start(out=xa[:, 0:W], in_=xT_sl[0:128, c0:c0 + W])
                nc.sync.dma_start(out=xb[:, 0:W], in_=xT_sl[128:256, c0:c0 + W])
                hTp = ps.tile([128, PW], F32, tag="pp")
                nc.tensor.matmul(
                    hTp[:, 0:W], lhsT=W1a[:], rhs=xa[:, 0:W], start=True, stop=False
                )
                nc.tensor.matmul(
                    hTp[:, 0:W], lhsT=W1b[:], rhs=xb[:, 0:W], start=False, stop=True
                )
                ha = sbx.tile([128, PW], F16, tag="ha")
                nc.scalar.activation(ha[:, 0:W], hTp[:, 0:W], AF.Relu, bias=b1c[:])
                hcn = sbx.tile([128, PW], F16, tag="hcn")
                nc.scalar.activation(
                    hcn[:, 0:W], hTp[:, 0:W], AF.Relu, bias=b1n[:], scale=-1.0
                )
                hdx = sbx.tile([128, PW], F16, tag="hdx")
                nc.scalar.activation(hdx[:, 0:W], hcn[:, 0:W], AF.Exp, scale=-1.0)
                h1T = sbx.tile([128, PW], F16, tag="h1T")
                nc.vector.scalar_tensor_tensor(
                    out=h1T[:, 0:W], in0=hdx[:, 0:W], scalar=-1.0, in1=ha[:, 0:W],
                    op0=AL.add, op1=AL.add,
                )
                hpTp = ps.tile([128, PW], F32, tag="pp")
                nc.tensor.matmul(
                    hpTp[:, 0:W], lhsT=R1[:], rhs=h1T[:, 0:W], start=True, stop=True
                )
                t1p = ps.tile([1, PW], F32, tag="pp")
                nc.tensor.matmul(
                    t1p[:, 0:W], lhsT=aw1t[:], rhs=h1T[:, 0:W], start=True, stop=True
                )
                t1row = sbx.tile([1, PW], F16, tag="t1row")
                nc.scalar.activation(
                    t1row[:, 0:W], t1p[:, 0:W], AF.Identity, bias=b1a[:]
                )
                ttp = ps.tile([128, 4], F32, tag="pp")
                for a in range(nbk):
                    nc.tensor.matmul(
                        ttp[:, a:a + 1], lhsT=t1row[0:1, a * 128:(a + 1) * 128],
                        rhs=ones1[:], start=True, stop=True, skip_group_check=True,
                    )
                nc.scalar.copy(t1T[:, p * 4:p * 4 + nbk], ttp[:, 0:nbk])
                hpT = sbx.tile([128, PW], F16, tag="hpT")
                nc.scalar.copy(hpT[:, 0:W], hpTp[:, 0:W])
                hr = sbx.tile([128, 4, 128], F16, tag="hr")
                for a in range(nbk):
                    trp = pst.tile([128, 128], F16, tag="tr")
                    nc.tensor.transpose(
                        trp[:], hpT[:, a * 128:(a + 1) * 128], I128[:]
                    )
                    nc.scalar.copy(hr[:, a, :], trp[:])
                    nc.sync.dma_start(
                        out=h1p_sl[c0 + a * 128:c0 + (a + 1) * 128, :],
                        in_=hr[:, a, :],
                    )

            nc.gpsimd.collective_compute(
                "AllGather",
                AL.bypass,
                replica_groups=[list(range(NCORES))],
                ins=[h1p_sl.ap().opt()],
                outs=[h1p_all.ap().opt()],
            )

            # ---------------- edge phase (both layers) ----------------
            def edge_layer(layer):
                if layer == 1:
                    table, d, ted, cc, Rm = h1p_all, HID_F, ted1, c1, R1
                else:
                    table, d, ted, cc, Rm = h2p_all, OUT_F, ted2, c2, R2

                emode = os.environ.get("GNN_EDGE", "full")
                for gidx, grp in enumerate(groups):
                    if layer == 1:
                        ted_emit(1, t1T, ted1, min(NB, grp["blocks"][-1] + 5))
                    nsg = grp["nsg"]
                    sg0 = grp["s0"]
                    gcalls = grp["calls"]
                    c_lo = gcalls[0][0]
                    c_hi = gcalls[-1][0] + gcalls[-1][1]
                    eib = sb.tile([128, NSGMAX * 8], I16, tag="eib", bufs=3)
                    nc.sync.dma_start(
                        out=eib[:, 0:c_hi - c_lo], in_=eidx[:, c_lo:c_hi]
                    )
                    oh = sb.tile([128, NSGMAX, 128], F8, tag="oh")
                    nc.sync.dma_start(
                        out=oh[:, 0:nsg, :],
                        in_=eoh[:, sg0 * 128:(sg0 + nsg) * 128],
                    )
                    g = sb3.tile([128, NSGMAX, 128], F16, tag="g", bufs=4)
                    for ci, (off, cols, n_idx, is_hi, slot0, cnt) in enumerate(
                        gcalls
                    ):
                        tbl_ap = table[TH:TBL, :] if is_hi else table[:, :]
                        gi = nc.gpsimd.dma_gather(
                            g[:, slot0:slot0 + cnt, :],
                            tbl_ap,
                            eib[:, off - c_lo:off - c_lo + cols],
                            num_idxs=n_idx,
                            num_idxs_reg=nidx_regs[n_idx],
                            elem_size=128,
                            single_packet=False,
                            queue_num=(2 * gidx + ci) % 4,
                        )
                        add_dep_helper(gi.ins, ll.ins)

                    if emode == "gather":
                        continue
                    # per-group edge math (small [128, nsg] ops)
                    z = sb.tile([128, NSGMAX], F16, tag="z")
                    nc.vector.scalar_tensor_tensor(
                        out=z[:, 0:nsg], in0=g[:, 0:nsg, 0], scalar=cc,
                        in1=ted[:, sg0:sg0 + nsg], op0=AL.mult, op1=AL.add,
                    )
                    zl = sb.tile([128, NSGMAX], F16, tag="zl")
                    nc.vector.scalar_tensor_tensor(
                        out=zl[:, 0:nsg], in0=z[:, 0:nsg], scalar=NEG,
                        in1=z[:, 0:nsg], op0=AL.mult, op1=AL.max,
                    )
                    zc = sb.tile([128, NSGMAX], F16, tag="zc")
                    nc.vector.scalar_tensor_tensor(
                        out=zc[:, 0:nsg], in0=zl[:, 0:nsg], scalar=0.0,
                        in1=clampc[:].to_broadcast([128, nsg]),
                        op0=AL.add, op1=AL.min,
                    )
                    gs = sb3.tile([128, NSGMAX, 132], F16, tag="gs", bufs=3)
                    nc.scalar.activation(
                        gs[:, 0:nsg, d], zc[:, 0:nsg], AF.Exp, bias=nsh[:]
                    )
                    nc.vector.scalar_tensor_tensor(
                        out=gs[:, 0:nsg, 0:d], in0=g[:, 0:nsg, 0:d], scalar=1.0,
                        in1=gs[:, 0:nsg, d].to_broadcast([128, nsg, d]),
                        op0=AL.mult, op1=AL.mult,
                    )
                    if emode == "nomm":
                        continue
                    for b in grp["blocks"]:
                        acc = psa.tile([128, 136], F32, tag="acc")
                        ranges = [r for r in (grp["lo_rng"][b], grp["hi_rng"][b])
                                  if r[1] > 0]
                        slots = [j for (off, cnt) in ranges
                                 for j in range(off, off + cnt)]
                        for ji, j in enumerate(slots):
                            nc.tensor.matmul(
                                acc[:, 0:d + 1], lhsT=oh[:, j, :],
                                rhs=gs[:, j, 0:d + 1],
                                start=(ji == 0), stop=(ji == len(slots) - 1),
                                skip_group_check=True,
                            )
                        edge_epilogue(layer, b, d, acc, Rm, emode)
                    if layer == 1 and emode == "full":
                        ted_emit(2, t2T, ted2, max(grp["blocks"]) + 1)

            def edge_epilogue(layer, b, d, acc, Rm, emode):
                    if emode == "noepi":
                        return
                    # -------- block epilogue --------
                    r0 = b * 128
                    den = sb.tile([128, 1], F32, tag="den")
                    nc.vector.tensor_scalar_add(den[:], acc[:, d:d + 1], EPS)
                    rec = sb.tile([128, 1], F32, tag="rec")
                    nc.vector.reciprocal(rec[:], den[:])
                    nrm = sb.tile([128, 128], F16, tag="nrm")
                    nc.vector.tensor_tensor(
                        out=nrm[:, 0:d], in0=acc[:, 0:d],
                        in1=rec[:].to_broadcast([128, d]), op=AL.mult,
                    )
                    nTp = pst.tile([d, 128], F16, tag="tr")
                    nc.tensor.transpose(nTp[:], nrm[:, 0:d], I128[:])
                    nT = sb.tile([d, 128], F16, tag="nT")
                    nc.scalar.copy(nT[:], nTp[:])
                    oTp = ps.tile([d, 128], F32, tag="pp")
                    nc.tensor.matmul(
                        oTp[:], lhsT=Rm[:], rhs=nT[:], start=True, stop=True
                    )

                    if layer == 1:
                        oT = sb.tile([128, 128], F16, tag="o1T")
                        nc.scalar.copy(oT[:], oTp[:])
                        h2Tp = ps.tile([64, 128], F32, tag="pp")
                        nc.tensor.matmul(
                            h2Tp[:], lhsT=W2[:], rhs=oT[:], start=True, stop=True
                        )
                        h2T = sb.tile([64, 128], F16, tag="h2T")
                        nc.scalar.activation(
                            h2T[:], h2Tp[:], AF.Identity, bias=b2c[:]
                        )
                        h2pTp = ps.tile([64, 128], F32, tag="pp")
                        nc.tensor.matmul(
                            h2pTp[:], lhsT=R2[:], rhs=h2T[:], start=True, stop=True
                        )
                        t2p = ps.tile([1, 128], F32, tag="pp")
                        nc.tensor.matmul(
                            t2p[:], lhsT=aw2t[:], rhs=h2T[:], start=True, stop=True
                        )
                        t2row = sb.tile([1, 128], F16, tag="t2row")
                        nc.scalar.activation(
                            t2row[:], t2p[:], AF.Identity, bias=b2a[:]
                        )
                        tt2 = ps.tile([128, 1], F32, tag="pp")
                        nc.tensor.matmul(
                            tt2[:], lhsT=t2row[:], rhs=ones1[:],
                            start=True, stop=True,
                        )
                        nc.scalar.copy(t2T[:, b:b + 1], tt2[:])
                        h2pT = sb.tile([64, 128], F16, tag="h2pT")
                        nc.scalar.copy(h2pT[:], h2pTp[:])
                        h2rp = pst.tile([128, 64], F16, tag="tr")
                        nc.tensor.transpose(h2rp[:], h2pT[:], I128[0:64, 0:64])
                        h2r = sb.tile([128, 128], F16, tag="h2r")
                        nc.scalar.copy(h2r[:, 0:64], h2rp[:])
                        nc.vector.memset(h2r[:, 64:128], 0.0)
                        nc.sync.dma_start(
                            out=h2p_sl[r0:r0 + 128, :], in_=h2r[:]
                        )
                    else:
                        o2T = sb.tile([64, 128], F16, tag="o2T")
                        nc.scalar.copy(o2T[:], oTp[:])
                        o2p = pst.tile([128, 64], F16, tag="tr")
                        nc.tensor.transpose(o2p[:], o2T[:], I128[0:64, 0:64])
                        o2 = sb.tile([128, 64], F32, tag="o2")
                        nc.scalar.copy(o2[:], o2p[:])
                        mx = sb.tile([128, 1], F32, tag="mx")
                        nc.vector.tensor_reduce(
                            out=mx[:], in_=o2[:], axis=AX.X, op=AL.max
                        )
                        mneg = sb.tile([128, 1], F32, tag="mneg")
                        nc.vector.tensor_scalar_mul(mneg[:], mx[:], -1.0)
                        ex = sb.tile([128, 64], F32, tag="ex")
                        nc.scalar.activation(ex[:], o2[:], AF.Exp, bias=mneg[:])
                        sm = sb.tile([128, 1], F32, tag="sm")
                        nc.vector.tensor_reduce(
                            out=sm[:], in_=ex[:], axis=AX.X, op=AL.add
                        )
                        ln = sb.tile([128, 1], F32, tag="ln")
                        nc.scalar.activation(ln[:], sm[:], AF.Ln)
                        mml = sb.tile([128, 1], F32, tag="mml")
                        nc.vector.tensor_tensor(
                            out=mml[:], in0=mx[:], in1=ln[:], op=AL.add
                        )
                        res = sb.tile([128, 64], F32, tag="res")
                        nc.vector.tensor_tensor(
                            out=res[:], in0=o2[:],
                            in1=mml[:].to_broadcast([128, 64]), op=AL.subtract,
                        )
                        nc.sync.dma_start(out=out_fin[r0:r0 + 128, :], in_=res[:])

            stop = os.environ.get("GNN_STOP", "full")
            if stop != "p1":
                ted_emit(1, t1T, ted1, 16)
            if stop in ("edge1", "full"):
                edge_layer(1)
            if stop == "full":
                nc.gpsimd.collective_compute(
                    "AllGather",
                    AL.bypass,
                    replica_groups=[list(range(NCORES))],
                    ins=[h2p_sl.ap().opt()],
                    outs=[h2p_all.ap().opt()],
                )
                ted_emit(2, t2T, ted2, NB)
                edge_layer(2)
            else:
                dbg = sb.tile([128, 64], F32, tag="dbgz")
                nc.vector.memset(dbg[:], 0.0)
                for bb in range(NB):
                    nc.sync.dma_start(out=out_fin[bb*128:(bb+1)*128, :], in_=dbg[:])

    return nc


_CACHE = {}


def kernel(**inputs):
    from concourse.bass_utils import run_bass_kernel_spmd
    from concourse.library_overlay import lower_extended_insts

    x = np.ascontiguousarray(np.asarray(inputs["x"], np.float32))
    ei = np.asarray(inputs["edge_index"])
    meta, eidx, OH8, OHT8 = prep_structures(ei)
    R1, c1 = _householder(np.asarray(inputs["a1_w"], np.float32)[:HID_F])
    R2, c2 = _householder(np.asarray(inputs["a2_w"], np.float32)[:OUT_F])
    consts = dict(
        W1=np.asarray(inputs["W1"], np.float32),
        b1=np.asarray(inputs["b1"], np.float32),
        W2=np.asarray(inputs["W2"], np.float32),
        b2=np.asarray(inputs["b2"], np.float32),
        a1_w=np.asarray(inputs["a1_w"], np.float32),
        a2_w=np.asarray(inputs["a2_w"], np.float32),
        a1_b=np.asarray(inputs["a1_b"], np.float32),
        a2_b=np.asarray(inputs["a2_b"], np.float32),
        R1=R1, R2=R2, c1=c1, c2=c2,
    )
    nc = build_bass(meta, consts)
    _split_multi_waits(nc)
    lower_extended_insts(nc)

    f8 = ml_dtypes.float8_e4m3fn
    in_maps = []
    for k in range(NCORES):
        xs = np.zeros((ROWS, IN_F), np.float32)
        xs[:NPC] = x[k * NPC:(k + 1) * NPC]
        in_maps.append(
            {
                "xT_sl": np.ascontiguousarray(xs.T.astype(np.float16)),
                "eidx": np.ascontiguousarray(eidx[k]),
                "eoh": OH8[k].view(f8),
                "eoht": OHT8[k].view(f8),
            }
        )

    trace = os.environ.get("GNN_TRACE", "0") == "1"
    if trace:
        try:
            import types
            from trn_agent_boot.trn_boot import _ntff_profile_via_ctypes
            _h = _ntff_profile_via_ctypes("/opt/axon/libaxon_pjrt.so")
            m = types.ModuleType("antenv.axon_hooks")
            m.get_axon_ntff_profile_hook = lambda: _h
            sys.modules["antenv.axon_hooks"] = m
        except Exception as e:
            print("profile hook setup failed:", e)
            trace = False
    res = run_bass_kernel_spmd(
        nc, in_maps, core_ids=list(range(NCORES)), trace=trace
    )
    kernel.last_results = res
    out = np.concatenate(
        [res.results[k]["out_fin"][:NPC] for k in range(NCORES)], axis=0
    )
    return out.astype(np.float32)


# revision 38
# speedup vs baseline: 1.0376x; 1.0376x over previous
import os
import sys

sys.path.insert(0, "/opt/trn_rl_repo")

import numpy as np
import ml_dtypes

# ---------------------------------------------------------------- problem dims
NCORES = 8
N = 50000
E = 800000
IN_F, HID_F, OUT_F = 256, 128, 64
NEG = 0.2
EPS = 1e-16

NPC = N // NCORES            # 6250 nodes (= targets) per core
BPB = 128                    # targets per block
NB = (NPC + BPB - 1) // BPB  # 49 blocks per core
ROWS = NB * BPB              # 6272 padded rows per core slice
TBL = NCORES * ROWS          # 50176 rows in the all-gathered table
TH = 32768                   # int16 gather index threshold
WSHIFT = 8.0                 # global exp shift (cancels in normalization)
ZCLAMP = 12.0                # safety clamp on zl (real zl stays < 5.2)
PW = 512                     # phase-1 pass width (node columns)


def _householder(a):
    """Symmetric orthogonal H with (H h)[0] == (a/||a||) . h ; returns H, ||a||."""
    a = np.asarray(a, dtype=np.float64)
    d = a.shape[0]
    alpha = np.linalg.norm(a)
    u = a.copy()
    sgn = 1.0 if a[0] >= 0 else -1.0
    u[0] += sgn * alpha
    nu = np.linalg.norm(u)
    Hm = np.eye(d) - 2.0 * np.outer(u, u) / (nu * nu)
    # H @ a = -sgn*alpha*e0  =>  (H h)[0] = -sgn * (a.h)/alpha; fold sign into c
    return Hm.astype(np.float32), np.float32(-sgn * alpha)


def prep_structures(edge_index, group=1):
    """Host-side layout. Blocks are grouped; per group one lo + one hi gather
    call covers all blocks in the group (slot order: los of each block, then
    his). Static fp8 one-hot (and transposed) matrices select target lanes."""
    src = edge_index[0].astype(np.int64)
    tgt = edge_index[1].astype(np.int64)
    adj = (src // NPC) * ROWS + (src % NPC)  # row in all-gathered table

    order = np.argsort(tgt, kind="stable")
    src_a = adj[order]
    tgt_s = tgt[order]

    core_of = tgt_s // NPC
    blk_of = (tgt_s % NPC) // BPB
    rel_of = (tgt_s % NPC) % BPB
    gb = core_of * NB + blk_of
    bounds = np.searchsorted(gb, np.arange(NCORES * NB + 1))

    lo_cnt = np.zeros((NCORES, NB), dtype=np.int64)
    hi_cnt = np.zeros((NCORES, NB), dtype=np.int64)
    per_kb = {}
    for k in range(NCORES):
        for b in range(NB):
            g = k * NB + b
            s, e = bounds[g], bounds[g + 1]
            sa = src_a[s:e]
            rl = rel_of[s:e]
            m = sa < TH
            lo_o = np.argsort(sa[m], kind="stable")
            hi_o = np.argsort(sa[~m], kind="stable")
            per_kb[(k, b)] = (sa[m][lo_o], rl[m][lo_o], sa[~m][hi_o], rl[~m][hi_o])
            lo_cnt[k, b] = int(m.sum())
            hi_cnt[k, b] = int((~m).sum())

    nlo = np.maximum(1, np.ceil(lo_cnt.max(axis=0) / 128.0)).astype(np.int64)
    nhi = np.ceil(hi_cnt.max(axis=0) / 128.0).astype(np.int64)  # may be 0

    NG = (NB + group - 1) // group
    groups = []      # per group: dict(blocks, s0, nsg, lo_rng{b}, hi_rng{b})
    s_off = 0
    SRC_cols = []    # per-core built later
    for gi in range(NG):
        blks = list(range(gi * group, min((gi + 1) * group, NB)))
        lo_rng = {}
        hi_rng = {}
        off = 0
        for b in blks:
            lo_rng[b] = (off, int(nlo[b]))
            off += int(nlo[b])
        lo_tot = off
        for b in blks:
            hi_rng[b] = (off, int(nhi[b]))
            off += int(nhi[b])
        groups.append(dict(blocks=blks, s0=s_off, nsg=off, lo_tot=lo_tot,
                           lo_rng=lo_rng, hi_rng=hi_rng))
        s_off += off
    S_TOT = s_off
    NSGMAX = max(g["nsg"] for g in groups)

    # per-core slot grids in the grouped order
    SRC = np.zeros((NCORES, 128, S_TOT), dtype=np.int64)
    REL = np.full((NCORES, 128, S_TOT), -1, dtype=np.int64)
    for k in range(NCORES):
        for g in groups:
            for b in g["blocks"]:
                la, lr, ha, hr = per_kb[(k, b)]
                for (arr, rel, rng) in ((la, lr, g["lo_rng"][b]),
                                        (ha, hr, g["hi_rng"][b])):
                    off, cnt = rng
                    if cnt == 0:
                        continue
                    base = g["s0"] + off
                    full = np.zeros(cnt * 128, dtype=np.int64)
                    full[:len(arr)] = arr
                    fr = np.full(cnt * 128, -1, dtype=np.int64)
                    fr[:len(arr)] = rel
                    SRC[k, :, base:base + cnt] = full.reshape(cnt, 128).T
                    REL[k, :, base:base + cnt] = fr.reshape(cnt, 128).T

    # per-call wrapped int16 index arrays (0-pads; counts uniform across cores)
    idx_parts = [[] for _ in range(NCORES)]
    col_off = 0
    for g in groups:
        bcalls = []
        for (is_hi, slot0, cnt) in ((False, 0, g["lo_tot"]),
                                    (True, g["lo_tot"], g["nsg"] - g["lo_tot"])):
            if cnt == 0:
                continue
            n_idx = cnt * 128
            cols = n_idx // 16
            bcalls.append((col_off, cols, n_idx, is_hi, slot0, cnt))
            col_off += cols
            for k in range(NCORES):
                s0 = g["s0"] + slot0
                vals = SRC[k][:, s0:s0 + cnt].flatten(order="F")
                if is_hi:
                    vals = np.maximum(vals - TH, 0)
                w16 = vals.reshape(-1, 16).T  # [16, cols]
                idx_parts[k].append(np.tile(w16, (8, 1)).astype(np.int16))
        g["calls"] = bcalls
    eidx = [np.concatenate(idx_parts[k], axis=1) for k in range(NCORES)]

    # fp8 one-hot grids (flattened [128, S_TOT*128] uint8; 1.0 == 0x38)
    ONE = np.uint8(0x38)
    tvec = np.arange(128, dtype=np.int64)
    OH8 = []
    OHT8 = []
    for k in range(NCORES):
        rel = REL[k]                      # [128 lanes, S_TOT]
        oh = (rel[:, :, None] == tvec[None, None, :])        # [lane, s, t]
        OH8.append(np.where(oh, ONE, np.uint8(0)).reshape(128, S_TOT * 128))
        oht = np.transpose(oh, (2, 1, 0))                     # [t, s, lane]
        OHT8.append(np.ascontiguousarray(
            np.where(oht, ONE, np.uint8(0)).reshape(128, S_TOT * 128)))

    meta = dict(
        groups=groups, S_TOT=S_TOT, NSGMAX=NSGMAX, TOT_COLS=col_off,
    )
    return meta, eidx, OH8, OHT8


# ------------------------------------------------------------------ bass build
def _patch_tile_drain():
    """This walrus build supports only one sync-wait per SP TPB_CTRL
    instruction; TileContext's exit drain aggregates the whole global clock
    onto one drain. Split each wait onto its own single-wait NOP first."""
    import concourse.mybir as mybir
    import concourse.tile as tile
    from concourse.tile import ScopedClock

    if getattr(tile.TileContext, "_drain_split_patched", False):
        return

    def _split(self, tick_clock, wait_clock):
        nop0 = self.nc.sync.nop()
        wait_clock.add_sem_waits(
            nop0.ins, ScopedClock({None: tick_clock.global_clock})
        )
        si = nop0.ins.sync_info
        if si is not None and si.on_wait and len(si.on_wait) > 1:
            waits = list(si.on_wait)
            nop0.ins.sync_info = mybir.SyncInfo(
                on_wait=[waits[0]], on_update=list(si.on_update)
            )
            for w in waits[1:]:
                n = self.nc.sync.nop()
                n.ins.sync_info = mybir.SyncInfo(on_wait=[w], on_update=[])
        self.nc.sync.drain()
        self.nc.all_engine_barrier()
        popped = self.nc._tile_sem_poison_stack.pop()
        assert popped is self._sem_poison
        self.nc.clear_and_free_semaphores(list(self.sems.allocated().values()))
        self.nc.all_engine_barrier()

    tile.TileContext._drain_and_barrier = _split
    tile.TileContext._drain_split_patched = True


def _split_multi_waits(nc):
    """This walrus build rejects instructions carrying more than one sync
    wait. Move extra waits onto single-wait NOPs inserted just before the
    instruction (same engine, same per-engine position)."""
    import concourse.mybir as mybir

    ctr = [0]
    for f in nc.m.functions:
        for bb in f.blocks:
            insts = list(bb.instructions)
            out = []
            changed = False
            for ins in insts:
                si = getattr(ins, "sync_info", None)
                if si is not None and si.on_wait and len(si.on_wait) > 1:
                    waits = list(si.on_wait)
                    for w in waits[:-1]:
                        n = mybir.InstNoOp(
                            name=f"splitw-{ctr[0]}", ins=[], outs=[]
                        )
                        ctr[0] += 1
                        n.engine = ins.engine
                        n.sync_info = mybir.SyncInfo(on_wait=[w], on_update=[])
                        nc.register_instruction(n)
                        out.append(n)
                    ins.sync_info = mybir.SyncInfo(
                        on_wait=[waits[-1]], on_update=list(si.on_update)
                    )
                    changed = True
                out.append(ins)
            if changed:
                bb.instructions = out


def build_bass(meta, consts):
    import concourse.bass as bass
    import concourse.mybir as mybir
    import concourse.tile as tile
    from concourse.library_config import mlp as mlp_lib
    from concourse.tile_rust import add_dep_helper

    _patch_tile_drain()

    F32 = mybir.dt.float32
    F16 = mybir.dt.float16
    F8 = mybir.dt.float8e4
    I16 = mybir.dt.int16
    AL = mybir.AluOpType
    AF = mybir.ActivationFunctionType
    AX = mybir.AxisListType
    S_TOT = meta["S_TOT"]
    NSGMAX = meta["NSGMAX"]
    groups = meta["groups"]

    nc = bass.Bass(num_devices=NCORES, num_swdge_queues=4)

    # per-core external inputs
    xT_sl = nc.dram_tensor("xT_sl", [IN_F, ROWS], F16, kind="ExternalInput")
    eidx = nc.dram_tensor(
        "eidx", [128, meta["TOT_COLS"]], I16, kind="ExternalInput"
    )
    eoh = nc.dram_tensor("eoh", [128, S_TOT * 128], F8, kind="ExternalInput")
    eoht = nc.dram_tensor("eoht", [128, S_TOT * 128], F8, kind="ExternalInput")
    out_fin = nc.dram_tensor("out_fin", [ROWS, OUT_F], F32, kind="ExternalOutput")

    def inl(name, arr):
        return nc.inline_tensor(np.ascontiguousarray(arr), name=name)

    f16 = np.float16
    c_W1a = inl("W1a", consts["W1"][:128].astype(f16))
    c_W1b = inl("W1b", consts["W1"][128:].astype(f16))
    c_R1 = inl("R1", consts["R1"].astype(f16))
    c_W2 = inl("W2", consts["W2"].astype(f16))
    c_R2 = inl("R2", consts["R2"].astype(f16))
    c_aw1t = inl("aw1t", consts["a1_w"][HID_F:].astype(f16).reshape(HID_F, 1))
    c_aw2t = inl("aw2t", consts["a2_w"][OUT_F:].astype(f16).reshape(OUT_F, 1))
    c_b1 = inl("b1c", consts["b1"].astype(np.float32).reshape(HID_F, 1))
    c_b1n = inl("b1n", (-consts["b1"]).astype(np.float32).reshape(HID_F, 1))
    c_b2 = inl("b2c", consts["b2"].astype(np.float32).reshape(OUT_F, 1))
    c_ones1 = inl("ones1", np.ones((1, 1), f16))
    c_onesr = inl("onesr", np.ones((1, 128), f16))
    c_I128 = inl("I128", np.eye(128, dtype=f16))
    c_b1a = inl("b1a", np.full((1, 1), consts["a1_b"][0], np.float32))
    c_b2a = inl("b2a", np.full((1, 1), consts["a2_b"][0], np.float32))
    c_nsh = inl("nsh", np.full((128, 1), -WSHIFT, np.float32))
    c_clamp = inl("clamp", np.full((128, 1), ZCLAMP, f16))
    c1 = float(consts["c1"])
    c2 = float(consts["c2"])

    # internal DRAM
    h1p_sl = nc.dram_tensor("h1p_sl", [ROWS, 128], F16)
    h1p_all = nc.dram_tensor("h1p_all", [TBL, 128], F16, addr_space="Shared")
    h2p_sl = nc.dram_tensor("h2p_sl", [ROWS, 128], F16)
    h2p_all = nc.dram_tensor("h2p_all", [TBL, 128], F16, addr_space="Shared")

    def mid_bcast(ap, cs):  # [128, X] -> [128, cs(bcast), X]
        return bass.AP(
            ap.tensor, ap.offset, [list(ap.ap[0]), [0, cs], list(ap.ap[1])]
        )

    with tile.TileContext(nc) as tc:
        import contextlib

        with contextlib.ExitStack() as ctx:
            cpool = ctx.enter_context(tc.tile_pool(name="consts", bufs=1))
            persist = ctx.enter_context(tc.tile_pool(name="persist", bufs=1))
            sbx = ctx.enter_context(tc.tile_pool(name="sbx", bufs=3))
            sb = ctx.enter_context(tc.tile_pool(name="sb", bufs=2))
            sb3 = ctx.enter_context(tc.tile_pool(name="sb3", bufs=3))
            ps = ctx.enter_context(tc.tile_pool(name="ps", bufs=4, space="PSUM"))
            psa = ctx.enter_context(tc.tile_pool(name="psa", bufs=2, space="PSUM"))
            pst = ctx.enter_context(tc.tile_pool(name="pst", bufs=2, space="PSUM"))

            def cload(handle, shape, dtype):
                t = cpool.tile(shape, dtype, tag=handle.name)
                nc.sync.dma_start(out=t[:], in_=handle[:, :])
                return t

            W1a = cload(c_W1a, [128, 128], F16)
            W1b = cload(c_W1b, [128, 128], F16)
            R1 = cload(c_R1, [128, 128], F16)
            W2 = cload(c_W2, [128, 64], F16)
            R2 = cload(c_R2, [64, 64], F16)
            aw1t = cload(c_aw1t, [128, 1], F16)
            aw2t = cload(c_aw2t, [64, 1], F16)
            b1c = cload(c_b1, [128, 1], F32)
            b1n = cload(c_b1n, [128, 1], F32)
            b2c = cload(c_b2, [64, 1], F32)
            ones1 = cload(c_ones1, [1, 1], F16)
            onesr = cload(c_onesr, [1, 128], F16)
            I128 = cload(c_I128, [128, 128], F16)
            b1a = cload(c_b1a, [1, 1], F32)
            b2a = cload(c_b2a, [1, 1], F32)
            nsh = cload(c_nsh, [128, 1], F32)
            clampc = cload(c_clamp, [128, 1], F16)

            t1T = persist.tile([128, NB], F16, tag="t1T")
            t2T = persist.tile([128, NB], F16, tag="t2T")
            ted1 = persist.tile([128, S_TOT], F16, tag="ted1")
            ted2 = persist.tile([128, S_TOT], F16, tag="ted2")

            ll = nc.gpsimd.load_library(mlp_lib)
            nidx_regs = {}
            for grp in groups:
                for (_o, _c, _n, _h, _s0, _cnt) in grp["calls"]:
                    if _n not in nidx_regs:
                        _r = nc.gpsimd.alloc_register(f"nidx_{_n}")
                        nc.gpsimd.reg_mov(_r, _n)
                        nidx_regs[_n] = _r

            # zero the gather-destination pool once (stale-lane safety)
            gz = []
            for i in range(4):
                t = sb3.tile([128, NSGMAX, 128], F16, tag="g", bufs=4)
                nc.vector.memset(t[:, :, :], 0.0)
                gz.append(t)

            # ---------------- t_ed precompute (per layer) ----------------
            ted_cursor = {1: 0, 2: 0}

            def ted_emit(which, tT, ted, max_block_excl):
                cur = ted_cursor[which]
                while (cur < len(groups)
                       and max(groups[cur]["blocks"]) < max_block_excl):
                    grp = groups[cur]
                    nsg = grp["nsg"]
                    sg0 = grp["s0"]
                    oht = sb.tile([128, NSGMAX, 128], F8, tag="oht")
                    nc.sync.dma_start(
                        out=oht[:, 0:nsg, :],
                        in_=eoht[:, sg0 * 128:(sg0 + nsg) * 128],
                    )
                    tp = ps.tile([128, NSGMAX], F32, tag="pp")
                    for b in grp["blocks"]:
                        for (off, cnt) in (grp["lo_rng"][b], grp["hi_rng"][b]):
                            for j in range(off, off + cnt):
                                nc.tensor.matmul(
                                    tp[:, j:j + 1], lhsT=oht[:, j, :],
                                    rhs=tT[:, b:b + 1], start=True, stop=True,
                                    skip_group_check=True,
                                )
                    nc.scalar.copy(ted[:, sg0:sg0 + nsg], tp[:, 0:nsg])
                    cur += 1
                ted_cursor[which] = cur


            # ---------------- phase 1: h1' table + t1 ----------------
            for p in range((ROWS + PW - 1) // PW):
                c0 = p * PW
                W = min(PW, ROWS - c0)
                nbk = W // 128
                xa = sbx.tile([128, PW], F16, tag="xa")
                xb = sbx.tile([128, PW], F16, tag="xb")
                nc.sync.dma_start(out=xa[:, 0:W], in_=xT_sl[0:128, c0:c0 + W])
                nc.sync.dma_start(out=xb[:, 0:W], in_=xT_sl[128:256, c0:c0 + W])
                hTp = ps.tile([128, PW], F32, tag="pp")
                nc.tensor.matmul(
                    hTp[:, 0:W], lhsT=W1a[:], rhs=xa[:, 0:W], start=True, stop=False
                )
                nc.tensor.matmul(
                    hTp[:, 0:W], lhsT=W1b[:], rhs=xb[:, 0:W], start=False, stop=True
                )
                ha = sbx.tile([128, PW], F16, tag="ha")
                nc.scalar.activation(ha[:, 0:W], hTp[:, 0:W], AF.Relu, bias=b1c[:])
                hcn = sbx.tile([128, PW], F16, tag="hcn")
                nc.scalar.activation(
                    hcn[:, 0:W], hTp[:, 0:W], AF.Relu, bias=b1n[:], scale=-1.0
                )
                hdx = sbx.tile([128, PW], F16, tag="hdx")
                nc.scalar.activation(hdx[:, 0:W], hcn[:, 0:W], AF.Exp, scale=-1.0)
                h1T = sbx.tile([128, PW], F16, tag="h1T")
                nc.vector.scalar_tensor_tensor(
                    out=h1T[:, 0:W], in0=hdx[:, 0:W], scalar=-1.0, in1=ha[:, 0:W],
                    op0=AL.add, op1=AL.add,
                )
                hpTp = ps.tile([128, PW], F32, tag="pp")
                nc.tensor.matmul(
                    hpTp[:, 0:W], lhsT=R1[:], rhs=h1T[:, 0:W], start=True, stop=True
                )
                t1p = ps.tile([1, PW], F32, tag="pp")
                nc.tensor.matmul(
                    t1p[:, 0:W], lhsT=aw1t[:], rhs=h1T[:, 0:W], start=True, stop=True
                )
                t1row = sbx.tile([1, PW], F16, tag="t1row")
                nc.scalar.activation(
                    t1row[:, 0:W], t1p[:, 0:W], AF.Identity, bias=b1a[:]
                )
                ttp = ps.tile([128, 4], F32, tag="pp")
                for a in range(nbk):
                    nc.tensor.matmul(
                        ttp[:, a:a + 1], lhsT=t1row[0:1, a * 128:(a + 1) * 128],
                        rhs=ones1[:], start=True, stop=True, skip_group_check=True,
                    )
                nc.scalar.copy(t1T[:, p * 4:p * 4 + nbk], ttp[:, 0:nbk])
                hpT = sbx.tile([128, PW], F16, tag="hpT")
                nc.scalar.copy(hpT[:, 0:W], hpTp[:, 0:W])
                hr = sbx.tile([128, 4, 128], F16, tag="hr")
                for a in range(nbk):
                    trp = pst.tile([128, 128], F16, tag="tr")
                    nc.tensor.transpose(
                        trp[:], hpT[:, a * 128:(a + 1) * 128], I128[:]
                    )
                    nc.scalar.copy(hr[:, a, :], trp[:])
                    nc.sync.dma_start(
                        out=h1p_sl[c0 + a * 128:c0 + (a + 1) * 128, :],
                        in_=hr[:, a, :],
                    )

            nc.gpsimd.collective_compute(
                "AllGather",
                AL.bypass,
                replica_groups=[list(range(NCORES))],
                ins=[h1p_sl.ap().opt()],
                outs=[h1p_all.ap().opt()],
            )

            # ---------------- edge phase (both layers) ----------------
            def edge_layer(layer):
                if layer == 1:
                    table, d, ted, cc, Rm = h1p_all, HID_F, ted1, c1, R1
                else:
                    table, d, ted, cc, Rm = h2p_all, OUT_F, ted2, c2, R2

                emode = os.environ.get("GNN_EDGE", "full")
                for gidx, grp in enumerate(groups):
                    if layer == 1:
                        ted_emit(1, t1T, ted1, min(NB, gidx + 4))
                    nsg = grp["nsg"]
                    sg0 = grp["s0"]
                    gcalls = grp["calls"]
                    c_lo = gcalls[0][0]
                    c_hi = gcalls[-1][0] + gcalls[-1][1]
                    eib = sb.tile([128, NSGMAX * 8], I16, tag="eib", bufs=3)
                    nc.sync.dma_start(
                        out=eib[:, 0:c_hi - c_lo], in_=eidx[:, c_lo:c_hi]
                    )
                    oh = sb.tile([128, NSGMAX, 128], F8, tag="oh")
                    nc.sync.dma_start(
                        out=oh[:, 0:nsg, :],
                        in_=eoh[:, sg0 * 128:(sg0 + nsg) * 128],
                    )
                    g = sb3.tile([128, NSGMAX, 128], F16, tag="g", bufs=4)
                    for ci, (off, cols, n_idx, is_hi, slot0, cnt) in enumerate(
                        gcalls
                    ):
                        tbl_ap = table[TH:TBL, :] if is_hi else table[:, :]
                        gi = nc.gpsimd.dma_gather(
                            g[:, slot0:slot0 + cnt, :],
                            tbl_ap,
                            eib[:, off - c_lo:off - c_lo + cols],
                            num_idxs=n_idx,
                            num_idxs_reg=nidx_regs[n_idx],
                            elem_size=128,
                            single_packet=False,
                            queue_num=(2 * gidx + ci) % 4,
                        )
                        add_dep_helper(gi.ins, ll.ins)

                    if emode == "gather":
                        continue
                    # per-group edge math (small [128, nsg] ops)
                    z = sb.tile([128, NSGMAX], F16, tag="z")
                    nc.vector.scalar_tensor_tensor(
                        out=z[:, 0:nsg], in0=g[:, 0:nsg, 0], scalar=cc,
                        in1=ted[:, sg0:sg0 + nsg], op0=AL.mult, op1=AL.add,
                    )
                    zl = sb.tile([128, NSGMAX], F16, tag="zl")
                    nc.vector.scalar_tensor_tensor(
                        out=zl[:, 0:nsg], in0=z[:, 0:nsg], scalar=NEG,
                        in1=z[:, 0:nsg], op0=AL.mult, op1=AL.max,
                    )
                    zc = sb.tile([128, NSGMAX], F16, tag="zc")
                    nc.vector.scalar_tensor_tensor(
                        out=zc[:, 0:nsg], in0=zl[:, 0:nsg], scalar=0.0,
                        in1=clampc[:].to_broadcast([128, nsg]),
                        op0=AL.add, op1=AL.min,
                    )
                    gs = sb3.tile([128, NSGMAX, 132], F16, tag="gs", bufs=3)
                    nc.scalar.activation(
                        gs[:, 0:nsg, d], zc[:, 0:nsg], AF.Exp, bias=nsh[:]
                    )
                    nc.vector.scalar_tensor_tensor(
                        out=gs[:, 0:nsg, 0:d], in0=g[:, 0:nsg, 0:d], scalar=1.0,
                        in1=gs[:, 0:nsg, d].to_broadcast([128, nsg, d]),
                        op0=AL.mult, op1=AL.mult,
                    )
                    if emode == "nomm":
                        continue
                    for b in grp["blocks"]:
                        acc = psa.tile([128, 136], F32, tag="acc")
                        ranges = [r for r in (grp["lo_rng"][b], grp["hi_rng"][b])
                                  if r[1] > 0]
                        slots = [j for (off, cnt) in ranges
                                 for j in range(off, off + cnt)]
                        for ji, j in enumerate(slots):
                            nc.tensor.matmul(
                                acc[:, 0:d + 1], lhsT=oh[:, j, :],
                                rhs=gs[:, j, 0:d + 1],
                                start=(ji == 0), stop=(ji == len(slots) - 1),
                                skip_group_check=True,
                            )
                        edge_epilogue(layer, b, d, acc, Rm, emode)
                    if layer == 1 and emode == "full":
                        ted_emit(2, t2T, ted2, max(grp["blocks"]) + 1)

            def edge_epilogue(layer, b, d, acc, Rm, emode):
                    if emode == "noepi":
                        return
                    # -------- block epilogue --------
                    r0 = b * 128
                    den = sb.tile([128, 1], F32, tag="den")
                    nc.vector.tensor_scalar_add(den[:], acc[:, d:d + 1], EPS)
                    rec = sb.tile([128, 1], F32, tag="rec")
                    nc.vector.reciprocal(rec[:], den[:])
                    nrm = sb.tile([128, 128], F16, tag="nrm")
                    nc.vector.tensor_tensor(
                        out=nrm[:, 0:d], in0=acc[:, 0:d],
                        in1=rec[:].to_broadcast([128, d]), op=AL.mult,
                    )
                    nTp = pst.tile([d, 128], F16, tag="tr")
                    nc.tensor.transpose(nTp[:], nrm[:, 0:d], I128[:])
                    nT = sb.tile([d, 128], F16, tag="nT")
                    nc.scalar.copy(nT[:], nTp[:])
                    oTp = ps.tile([d, 128], F32, tag="pp")
                    nc.tensor.matmul(
                        oTp[:], lhsT=Rm[:], rhs=nT[:], start=True, stop=True
                    )

                    if layer == 1:
                        oT = sb.tile([128, 128], F16, tag="o1T")
                        nc.scalar.copy(oT[:], oTp[:])
                        h2Tp = ps.tile([64, 128], F32, tag="pp")
                        nc.tensor.matmul(
                            h2Tp[:], lhsT=W2[:], rhs=oT[:], start=True, stop=True
                        )
                        h2T = sb.tile([64, 128], F16, tag="h2T")
                        nc.scalar.activation(
                            h2T[:], h2Tp[:], AF.Identity, bias=b2c[:]
                        )
                        h2pTp = ps.tile([64, 128], F32, tag="pp")
                        nc.tensor.matmul(
                            h2pTp[:], lhsT=R2[:], rhs=h2T[:], start=True, stop=True
                        )
                        t2p = ps.tile([1, 128], F32, tag="pp")
                        nc.tensor.matmul(
                            t2p[:], lhsT=aw2t[:], rhs=h2T[:], start=True, stop=True
                        )
                        t2row = sb.tile([1, 128], F16, tag="t2row")
                        nc.scalar.activation(
                            t2row[:], t2p[:], AF.Identity, bias=b2a[:]
                        )
                        tt2 = ps.tile([128, 1], F32, tag="pp")
                        nc.tensor.matmul(
                            tt2[:], lhsT=t2row[:], rhs=ones1[:],
                            start=True, stop=True,
                        )
                        nc.scalar.copy(t2T[:, b:b + 1], tt2[:])
                        h2pT = sb.tile([64, 128], F16, tag="h2pT")
                        nc.scalar.copy(h2pT[:], h2pTp[:])
                        h2rp = pst.tile([128, 64], F16, tag="tr")
                        nc.tensor.transpose(h2rp[:], h2pT[:], I128[0:64, 0:64])
                        h2r = sb.tile([128, 128], F16, tag="h2r")
                        nc.scalar.copy(h2r[:, 0:64], h2rp[:])
                        nc.vector.memset(h2r[:, 64:128], 0.0)
                        nc.sync.dma_start(
                            out=h2p_sl[r0:r0 + 128, :], in_=h2r[:]
                        )
                    else:
                        o2T = sb.tile([64, 128], F16, tag="o2T")
                        nc.scalar.copy(o2T[:], oTp[:])
                        o2p = pst.tile([128, 64], F16, tag="tr")
                        nc.tensor.transpose(o2p[:], o2T[:], I128[0:64, 0:64])
                        o2 = sb.tile([128, 64], F32, tag="o2")
                        nc.scalar.copy(o2[:], o2p[:])
                        mx = sb.tile([128, 1], F32, tag="mx")
                        nc.vector.tensor_reduce(
                            out=mx[:], in_=o2[:], axis=AX.X, op=AL.max
                        )
                        mneg = sb.tile([128, 1], F32, tag="mneg")
                        nc.vector.tensor_scalar_mul(mneg[:], mx[:], -1.0)
                        ex = sb.tile([128, 64], F32, tag="ex")
                        nc.scalar.activation(ex[:], o2[:], AF.Exp, bias=mneg[:])
                        sm = sb.tile([128, 1], F32, tag="sm")
                        nc.vector.tensor_reduce(
                            out=sm[:], in_=ex[:], axis=AX.X, op=AL.add
                        )
                        ln = sb.tile([128, 1], F32, tag="ln")
                        nc.scalar.activation(ln[:], sm[:], AF.Ln)
                        mml = sb.tile([128, 1], F32, tag="mml")
                        nc.vector.tensor_tensor(
                            out=mml[:], in0=mx[:], in1=ln[:], op=AL.add
                        )
                        res = sb.tile([128, 64], F32, tag="res")
                        nc.vector.tensor_tensor(
                            out=res[:], in0=o2[:],
                            in1=mml[:].to_broadcast([128, 64]), op=AL.subtract,
                        )
                        nc.sync.dma_start(out=out_fin[r0:r0 + 128, :], in_=res[:])

            stop = os.environ.get("GNN_STOP", "full")
            if stop != "p1":
                ted_emit(1, t1T, ted1, 16)
            if stop in ("edge1", "full"):
                edge_layer(1)
            if stop == "full":
                nc.gpsimd.collective_compute(
                    "AllGather",
                    AL.bypass,
                    replica_groups=[list(range(NCORES))],
                    ins=[h2p_sl.ap().opt()],
                    outs=[h2p_all.ap().opt()],
                )
                ted_emit(2, t2T, ted2, NB)
                edge_layer(2)
            else:
                dbg = sb.tile([128, 64], F32, tag="dbgz")
                nc.vector.memset(dbg[:], 0.0)
                for bb in range(NB):
                    nc.sync.dma_start(out=out_fin[bb*128:(bb+1)*128, :], in_=dbg[:])

    return nc


_CACHE = {}


def kernel(**inputs):
    from concourse.bass_utils import run_bass_kernel_spmd
    from concourse.library_overlay import lower_extended_insts

    x = np.ascontiguousarray(np.asarray(inputs["x"], np.float32))
    ei = np.asarray(inputs["edge_index"])
    meta, eidx, OH8, OHT8 = prep_structures(ei)
    R1, c1 = _householder(np.asarray(inputs["a1_w"], np.float32)[:HID_F])
    R2, c2 = _householder(np.asarray(inputs["a2_w"], np.float32)[:OUT_F])
    consts = dict(
        W1=np.asarray(inputs["W1"], np.float32),
        b1=np.asarray(inputs["b1"], np.float32),
        W2=np.asarray(inputs["W2"], np.float32),
        b2=np.asarray(inputs["b2"], np.float32),
        a1_w=np.asarray(inputs["a1_w"], np.float32),
        a2_w=np.asarray(inputs["a2_w"], np.float32),
        a1_b=np.asarray(inputs["a1_b"], np.float32),
        a2_b=np.asarray(inputs["a2_b"], np.float32),
        R1=R1, R2=R2, c1=c1, c2=c2,
    )
    nc = build_bass(meta, consts)
    _split_multi_waits(nc)
    lower_extended_insts(nc)

    f8 = ml_dtypes.float8_e4m3fn
    in_maps = []
    for k in range(NCORES):
        xs = np.zeros((ROWS, IN_F), np.float32)
        xs[:NPC] = x[k * NPC:(k + 1) * NPC]
        in_maps.append(
            {
                "xT_sl": np.ascontiguousarray(xs.T.astype(np.float16)),
                "eidx": np.ascontiguousarray(eidx[k]),
                "eoh": OH8[k].view(f8),
                "eoht": OHT8[k].view(f8),
            }
        )

    trace = os.environ.get("GNN_TRACE", "0") == "1"
    if trace:
        try:
            import types
            from trn_agent_boot.trn_boot import _ntff_profile_via_ctypes
            _h = _ntff_profile_via_ctypes("/opt/axon/libaxon_pjrt.so")
            m = types.ModuleType("antenv.axon_hooks")
            m.get_axon_ntff_profile_hook = lambda: _h
            sys.modules["antenv.axon_hooks"] = m
        except Exception as e:
            print("profile hook setup failed:", e)
            trace = False
    res = run_bass_kernel_spmd(
        nc, in_maps, core_ids=list(range(NCORES)), trace=trace
    )
    kernel.last_results = res
    out = np.concatenate(
        [res.results[k]["out_fin"][:NPC] for k in range(NCORES)], axis=0
    )
    return out.astype(np.float32)
